# revision 2
# baseline (speedup 1.0000x reference)
"""Trainium2 Bass kernel for nn_EuclideanToLorentzConv (8-core data-parallel).

Current best (v6 lineage): rt stored via the sync HWDGE queue (the gpsimd SWDGE path spent
~25us of Q7 descriptor generation on it at the kernel tail) and the T row of
the phase-3 staging buffer filled before the first AllReduce.
"""

import sys
import numpy as np
from contextlib import ExitStack

sys.path.insert(0, "/opt/trn_rl_repo")

import concourse.bass as bass  # noqa: E402
import concourse.tile as tile  # noqa: E402
from concourse import mybir, bacc  # noqa: E402
from concourse.bass_utils import run_bass_kernel_spmd  # noqa: E402

import ml_dtypes  # noqa: E402

F32 = mybir.dt.float32
BF16 = mybir.dt.bfloat16
AX = mybir.AxisListType
OP = mybir.AluOpType
AF = mybir.ActivationFunctionType

# ---- problem constants (hardcoded; kernel.py must be self-contained) ----
NCORES = 8
B_GLOB, CIN, H, W = 16, 64, 112, 112
B_LOC = B_GLOB // NCORES            # 2 images per core
S = CIN - 1                         # 63 space channels in
M = 127                             # space channels out
COUT = M + 1
D = 9 * S + 1                       # 568
EPS = 1e-6

HP, WP = H + 2, W + 2               # padded
ROWS_PER_GROUP = 4
GROUP_PX = ROWS_PER_GROUP * W       # 448
BAND_ROWS = 16                      # output rows per band
GROUPS_PER_BAND = BAND_ROWS // ROWS_PER_GROUP   # 4
BANDS_PER_IMG = H // BAND_ROWS      # 7
NBANDS = B_LOC * BANDS_PER_IMG      # 14
NGROUPS = NBANDS * GROUPS_PER_BAND  # 56
NPX = NGROUPS * GROUP_PX            # 25088 pixels per core
NPX_GLOB = B_GLOB * H * W           # 200704
SPAD_ROWS = BAND_ROWS + 2           # 18 padded rows per band buffer

_CACHE = {}


def _build_nc():
    nc = bacc.Bacc("TRN2", target_bir_lowering=False, debug=False,
                   num_devices=NCORES)

    # host-padded inputs: 64 channels (ch 63 zero), 114x114 spatial pad
    xb_in = nc.dram_tensor("xb", [B_LOC, 64, HP, WP], BF16, kind="ExternalInput")
    xq_in = nc.dram_tensor("xq", [B_LOC, 64, HP, WP], BF16, kind="ExternalInput")
    w9_in = nc.dram_tensor("w9", [128, 9 * 128], BF16, kind="ExternalInput")
    redw_in = nc.dram_tensor("redw", [M, 3], F32, kind="ExternalInput")
    redwb_in = nc.dram_tensor("redwb", [M, 3], BF16, kind="ExternalInput")
    lr1_in = nc.dram_tensor("lr1i", [2, 128], BF16, kind="ExternalInput")
    gamma_in = nc.dram_tensor("gamma", [1], F32, kind="ExternalInput")
    out_d = nc.dram_tensor("out", [B_LOC, COUT, H, W], BF16,
                           kind="ExternalOutput")

    cc1_in = nc.dram_tensor("cc1_in", [130], F32)
    cc1_out = nc.dram_tensor("cc1_out", [130], F32, addr_space="Shared")
    cc2_in = nc.dram_tensor("cc2_in", [2], F32)
    cc2_out = nc.dram_tensor("cc2_out", [2], F32, addr_space="Shared")
    groups_all = [list(range(NCORES))]

    with tile.TileContext(nc) as tc, ExitStack() as ctx:
        sing = ctx.enter_context(tc.tile_pool(name="sing", bufs=1))
        spadp = ctx.enter_context(tc.tile_pool(name="spad", bufs=3))
        scrp = ctx.enter_context(tc.tile_pool(name="scr", bufs=3))
        tmpp = ctx.enter_context(tc.tile_pool(name="tmpp", bufs=3))
        outp = ctx.enter_context(tc.tile_pool(name="outp", bufs=8))
        stgp = ctx.enter_context(tc.tile_pool(name="stg", bufs=2))
        psy = ctx.enter_context(tc.tile_pool(name="psy", bufs=4, space="PSUM"))
        pss = ctx.enter_context(tc.tile_pool(name="pss", bufs=3, space="PSUM"))
        pst = ctx.enter_context(tc.tile_pool(name="pst", bufs=1, space="PSUM"))

        # ---- static SBUF ----
        W9 = sing.tile([128, 9, 128], BF16)
        nc.sync.dma_start(out=W9, in_=w9_in[:].rearrange("p (w m) -> p w m", w=9))
        REDW = sing.tile([M, 3], F32)
        nc.sync.dma_start(out=REDW, in_=redw_in[:])
        REDWB = sing.tile([M, 3], BF16)
        nc.sync.dma_start(out=REDWB, in_=redwb_in[:])
        LR1B = sing.tile([2, 128], BF16)
        nc.sync.dma_start(out=LR1B, in_=lr1_in[:])
        GAM = sing.tile([1, 1], F32)
        nc.sync.dma_start(out=GAM, in_=gamma_in[:].rearrange("(o c) -> o c", o=1))
        ONES56 = sing.tile([56, 1], F32)
        nc.vector.memset(ONES56, 1.0)
        ONESB = sing.tile([M, 1], BF16)
        nc.vector.memset(ONESB, 1.0)
        ONEROW = sing.tile([33, M], BF16)   # lhsT for w2 row-broadcast (row 32)
        nc.vector.memset(ONEROW[32:33, :], 1.0)
        BYT = sing.tile([56, 1], F32)
        nc.vector.memset(BYT, float(1.0 + _CACHE["c_w0sq"]))
        BM1 = sing.tile([56, 1], F32)
        nc.vector.memset(BM1, -1.0)
        BEPSV = sing.tile([1, 1], F32)
        nc.vector.memset(BEPSV, 1e-5)

        YCM = sing.tile([128, NPX], BF16)         # rows 0..126 y', row 127 T^2-1
        MUP = sing.tile([128, NGROUPS], F32)      # per-group per-channel sums
        # rows 0-1: H,T (rank-2 matmul rhs); row 32: w2 (base_partition rule)
        HTW = sing.tile([33, NPX], BF16)

        # pixel-scalar fields, [56, 448] (partition = group)
        def ps(name, dt=F32):
            return sing.tile([NGROUPS, GROUP_PX], dt, tag=name, name=name)
        T2M1 = ps("t2m1", BF16)
        TPS, W0DOT, YSQ1, YT = ps("tps"), ps("w0dot"), ps("ysq1"), ps("yt")
        MUDOT, ALPHA, FPS, HPS = ps("mudot"), ps("alpha"), ps("fps"), ps("hps")
        STSQ, RSQ2, PSA, PSB, PSC = ps("stsq"), ps("rsq2"), ps("psa"), ps("psb"), ps("psc")
        HPB, TPB, W2B = ps("hpb", BF16), ps("tpb", BF16), ps("w2b", BF16)

        # ================= PHASE 1: conv =================
        for band in range(NBANDS):
            b, rb = divmod(band, BANDS_PER_IMG)
            r0p = rb * BAND_ROWS                 # first padded input row
            SPAD = spadp.tile([128, SPAD_ROWS, WP], BF16, tag="spad")
            nc.sync.dma_start(out=SPAD[0:64, :, :],
                              in_=xb_in[b, :, r0p:r0p + SPAD_ROWS, :])
            nc.sync.dma_start(out=SPAD[64:128, :, :],
                              in_=xq_in[b, :, r0p:r0p + SPAD_ROWS, :])

            STG_AB = stgp.tile([2, GROUPS_PER_BAND, GROUP_PX], F32, tag="stgx")
            for k in range(GROUPS_PER_BAND):
                g = band * GROUPS_PER_BAND + k
                cols = bass.ts(g, GROUP_PX)
                R = k * ROWS_PER_GROUP                  # band-local out row
                psum = psy.tile([128, GROUP_PX], F32, tag="psy")
                for wi in range(9):
                    i, j = divmod(wi, 3)
                    rhs = SPAD[:, R + i:R + i + ROWS_PER_GROUP, j:j + W]
                    nc.tensor.matmul(psum[:], lhsT=W9[:, wi, :], rhs=rhs,
                                     start=(wi == 0), stop=(wi == 8))
                # evacuate (bf16) + per-channel partial sums (fp32, for mu)
                nc.vector.tensor_scalar(out=YCM[:, cols], in0=psum[:],
                                        scalar1=0.0, scalar2=None, op0=OP.add,
                                        op1=OP.add, accum_out=MUP[:, g:g + 1])
                # squared copy for sum_c y'^2
                ysq_t = scrp.tile([M, GROUP_PX], BF16, tag="ysqscr")
                nc.scalar.activation(out=ysq_t, in_=psum[0:M, :], func=AF.Square)
                ps2 = pss.tile([2, GROUP_PX], F32, tag="pss")
                nc.tensor.matmul(ps2[0:2, :], lhsT=REDWB[:, 0:2],
                                 rhs=YCM[0:M, cols], start=True, stop=False)
                nc.tensor.matmul(ps2[0:2, :], lhsT=REDWB[:, 1:3],
                                 rhs=ysq_t[:], start=False, stop=True)
                nc.vector.tensor_copy(out=STG_AB[:, k, :], in_=ps2[:])
            gsl = bass.ts(band, GROUPS_PER_BAND)
            csl = bass.ts(band, GROUPS_PER_BAND * GROUP_PX)
            nc.sync.dma_start(out=W0DOT[gsl, :], in_=STG_AB[0:1, :, :])
            nc.sync.dma_start(out=YSQ1[gsl, :], in_=STG_AB[1:2, :, :])
            nc.sync.dma_start(out=T2M1[gsl, :], in_=YCM[127:128, csl])

        # ---- pixel-scalar chain, phase 1 ----
        # T = sqrt(1 + T2m1)
        nc.scalar.activation(out=TPS, in_=T2M1, func=AF.Sqrt, bias=1.0)
        # ysqf = ysq1 + 2*T*w0dot + T2m1*c_w0sq ; y_t = sqrt(1 + c_w0sq + ysqf')
        nc.vector.tensor_mul(PSA, TPS, W0DOT)
        nc.vector.scalar_tensor_tensor(out=PSB, in0=PSA, scalar=2.0, in1=YSQ1,
                                       op0=OP.mult, op1=OP.add)
        nc.vector.scalar_tensor_tensor(out=PSC, in0=T2M1, scalar=_CACHE["c_w0sq"],
                                       in1=PSB, op0=OP.mult, op1=OP.add)
        nc.scalar.activation(out=YT, in_=PSC, func=AF.Sqrt, bias=BYT[:])
        # T is final already: cast + stage its phase-3 matmul row now,
        # overlapping the first AllReduce
        nc.vector.tensor_scalar_mul(TPB, TPS, 1.0)
        nc.sync.dma_start(out=HTW[1:2, :], in_=TPB[:])
        # reduced sums for the collective
        MUS = sing.tile([128, 1], F32)
        nc.vector.tensor_reduce(MUS, MUP, axis=AX.X, op=OP.add)
        SR = sing.tile([56, 2], F32)
        nc.vector.tensor_reduce(SR[:, 0:1], TPS, axis=AX.X, op=OP.add)
        nc.vector.tensor_reduce(SR[:, 1:2], YT, axis=AX.X, op=OP.add)
        pt = pst.tile([1, 8], F32, tag="pst")
        nc.tensor.matmul(pt[0:1, 0:2], lhsT=ONES56, rhs=SR[:], start=True, stop=True)
        SC0 = sing.tile([1, 2], F32)
        nc.vector.tensor_copy(out=SC0, in_=pt[0:1, 0:2])
        nc.sync.dma_start(out=cc1_in[0:128], in_=MUS)
        nc.sync.dma_start(out=cc1_in[128:130], in_=SC0)
        nc.gpsimd.collective_compute("AllReduce", OP.add, replica_groups=groups_all,
                                     ins=[cc1_in[:]], outs=[cc1_out[:]])
        MUSG = sing.tile([128, 1], F32)
        nc.sync.dma_start(out=MUSG, in_=cc1_out[0:128].rearrange("(p o) -> p o", o=1))
        SC0G = sing.tile([1, 2], F32)
        nc.sync.dma_start(out=SC0G, in_=cc1_out[128:130].rearrange("(o c) -> o c", o=1))

        # ---- mu normalization (tiny ops) ----
        invN = 1.0 / float(NPX_GLOB)
        SC127 = sing.tile([M, 2], F32)
        nc.gpsimd.partition_broadcast(SC127, SC0G)
        MUUS = sing.tile([M, 1], F32)      # unnormalized mean of y_s
        nc.vector.scalar_tensor_tensor(out=MUUS, in0=REDW[:, 0:1],
                                       scalar=SC127[:, 0:1], in1=MUSG[0:M, :],
                                       op0=OP.mult, op1=OP.add)
        nc.vector.tensor_scalar_mul(MUUS, MUUS, invN)
        MU0U = sing.tile([1, 1], F32)
        nc.vector.tensor_scalar_mul(MU0U, SC0G[0:1, 1:2], invN)
        MSQ = sing.tile([M, 1], F32)
        nc.vector.tensor_mul(MSQ, MUUS, MUUS)
        pt2 = pst.tile([1, 8], F32, tag="pst")
        nc.tensor.matmul(pt2[0:1, 0:1], lhsT=REDW[:, 2:3], rhs=MSQ[:],
                         start=True, stop=True)
        SMSQ = sing.tile([1, 1], F32)
        nc.vector.tensor_copy(out=SMSQ, in_=pt2[0:1, 0:1])
        T1 = sing.tile([1, 1], F32)
        nc.vector.tensor_mul(T1, MU0U, MU0U)
        nc.vector.tensor_sub(T1, T1, SMSQ)
        nc.scalar.activation(out=T1, in_=T1, func=AF.Sqrt)     # nrm
        RNRM = sing.tile([1, 1], F32)
        nc.vector.reciprocal(RNRM, T1)
        RN127 = sing.tile([M, 1], F32)
        nc.gpsimd.partition_broadcast(RN127, RNRM)
        MUHS = sing.tile([M, 1], F32)
        nc.vector.tensor_scalar_mul(MUHS, MUUS, RN127[:, 0:1])
        MUHSB = sing.tile([M, 1], BF16)
        nc.vector.tensor_scalar_mul(MUHSB, MUHS, 1.0)
        MU0H = sing.tile([1, 1], F32)
        nc.vector.tensor_mul(MU0H, MU0U, RNRM)
        # c_musq = |mu_s|^2 = SMSQ * rnrm^2
        CMSQ = sing.tile([1, 1], F32)
        nc.vector.tensor_mul(CMSQ, SMSQ, RNRM)
        nc.vector.tensor_mul(CMSQ, CMSQ, RNRM)
        # c_muW0 = sum(mu_s * W0)
        PRD = sing.tile([M, 1], F32)
        nc.vector.tensor_mul(PRD, MUHS, REDW[:, 0:1])
        pt3 = pst.tile([1, 8], F32, tag="pst")
        nc.tensor.matmul(pt3[0:1, 0:1], lhsT=REDW[:, 2:3], rhs=PRD[:],
                         start=True, stop=True)
        # inv1p = 1/(1+mu0)
        INV1P = sing.tile([1, 1], F32)
        nc.vector.tensor_scalar_add(INV1P, MU0H, 1.0)
        nc.vector.reciprocal(INV1P, INV1P)
        # scalar bundle -> 56 partitions: {mu0, inv1p, c_muW0, c_musq}
        SCROW = sing.tile([1, 4], F32)
        nc.vector.tensor_copy(out=SCROW[:, 0:1], in_=MU0H)
        nc.vector.tensor_copy(out=SCROW[:, 1:2], in_=INV1P)
        nc.vector.tensor_copy(out=SCROW[:, 2:3], in_=pt3[0:1, 0:1])
        nc.vector.tensor_copy(out=SCROW[:, 3:4], in_=CMSQ)
        SC56 = sing.tile([56, 4], F32)
        nc.gpsimd.partition_broadcast(SC56, SCROW)
        # LR1B row0 = -mu_s (bf16; tiny transposing DMA [127,1] -> [1,127])
        NMUB = sing.tile([M, 1], BF16)
        nc.vector.tensor_scalar_mul(NMUB, MUHS, -1.0)
        nc.sync.dma_start(out=LR1B[0:1, 0:M], in_=NMUB[:])

        # ================= PHASE 2: mudot only =================
        for band in range(NBANDS):
            STG_C = stgp.tile([1, GROUPS_PER_BAND, GROUP_PX], F32, tag="stgc")
            for k in range(GROUPS_PER_BAND):
                g = band * GROUPS_PER_BAND + k
                cols = bass.ts(g, GROUP_PX)
                ps2 = pss.tile([2, GROUP_PX], F32, tag="pss")
                nc.tensor.matmul(ps2[0:1, :], lhsT=MUHSB, rhs=YCM[0:M, cols],
                                 start=True, stop=True)
                nc.vector.tensor_copy(out=STG_C[:, k, :], in_=ps2[0:1, :])
            nc.sync.dma_start(out=MUDOT[bass.ts(band, GROUPS_PER_BAND), :], in_=STG_C[0:1, :, :])

        # alpha = clip(mu0*yt - (mudot + T*c_muW0), 1+eps)
        nc.vector.scalar_tensor_tensor(out=PSA, in0=TPS, scalar=SC56[:, 2:3],
                                       in1=MUDOT, op0=OP.mult, op1=OP.add)
        nc.vector.tensor_scalar(out=PSB, in0=YT, scalar1=SC56[:, 0:1],
                                scalar2=None, op0=OP.mult)
        nc.vector.tensor_sub(ALPHA, PSB, PSA)
        nc.vector.tensor_scalar_max(ALPHA, ALPHA, 1.0 + EPS)
        # f = ln(alpha + sqrt(alpha^2-1)) / sqrt(alpha^2-1)
        nc.vector.tensor_mul(PSB, ALPHA, ALPHA)
        nc.scalar.activation(out=PSB, in_=PSB, func=AF.Sqrt, bias=BM1[:])
        nc.vector.tensor_add(FPS, ALPHA, PSB)
        nc.scalar.activation(out=FPS, in_=FPS, func=AF.Ln)
        nc.vector.reciprocal(PSB, PSB)
        nc.vector.tensor_mul(FPS, FPS, PSB)
        # H = alpha + (yt - alpha*mu0) * inv1p
        nc.vector.tensor_scalar(out=PSB, in0=ALPHA, scalar1=SC56[:, 0:1],
                                scalar2=None, op0=OP.mult)
        nc.vector.tensor_sub(PSB, YT, PSB)
        nc.vector.scalar_tensor_tensor(out=HPS, in0=PSB, scalar=SC56[:, 1:2],
                                       in1=ALPHA, op0=OP.mult, op1=OP.add)
        # bf16 copy of H, staged to the matmul rhs layout (T staged earlier)
        nc.vector.tensor_scalar_mul(HPB, HPS, 1.0)
        nc.sync.dma_start(out=HTW[0:1, :], in_=HPB[:])
        # stsq = (PSC + c_w0sq) + H*(H*c_musq - 2*(mudot + T*c_muW0))
        nc.vector.tensor_scalar(out=PSB, in0=HPS, scalar1=SC56[:, 3:4],
                                scalar2=None, op0=OP.mult)
        nc.vector.scalar_tensor_tensor(out=PSB, in0=PSA, scalar=-2.0,
                                       in1=PSB, op0=OP.mult, op1=OP.add)
        nc.vector.tensor_mul(PSB, PSB, HPS)
        nc.vector.scalar_tensor_tensor(out=STSQ, in0=PSC,
                                       scalar=_CACHE["c_w0sq"],
                                       in1=PSB, op0=OP.add, op1=OP.add)

        # tmp = y' + (W0*T - mu_s*H), accumulated into YCM in place.
        # Independent of w2/var, so it overlaps the second AllReduce.
        for g in range(NGROUPS):
            cols = bass.ts(g, GROUP_PX)
            pr1 = psy.tile([128, GROUP_PX], F32, tag="psy")
            nc.tensor.matmul(pr1[:], lhsT=LR1B, rhs=HTW[0:2, cols],
                             start=True, stop=True)
            nc.vector.scalar_tensor_tensor(out=YCM[0:M, cols],
                                           in0=YCM[0:M, cols],
                                           scalar=1.0, in1=pr1[0:M, :],
                                           op0=OP.mult, op1=OP.add)

        # var = mean(f^2 * stsq)  -> allreduce
        nc.vector.tensor_mul(PSA, FPS, FPS)
        nc.vector.tensor_mul(PSB, PSA, STSQ)
        VR = sing.tile([56, 1], F32)
        nc.vector.tensor_reduce(VR, PSB, axis=AX.X, op=OP.add)
        pt4 = pst.tile([1, 8], F32, tag="pst")
        nc.tensor.matmul(pt4[0:1, 0:1], lhsT=ONES56, rhs=VR[:], start=True, stop=True)
        VSC = sing.tile([1, 2], F32)
        nc.vector.tensor_copy(out=VSC[:, 0:1], in_=pt4[0:1, 0:1])
        nc.vector.tensor_copy(out=VSC[:, 1:2], in_=pt4[0:1, 0:1])
        nc.sync.dma_start(out=cc2_in[:], in_=VSC)
        nc.gpsimd.collective_compute("AllReduce", OP.add, replica_groups=groups_all,
                                     ins=[cc2_in[:]], outs=[cc2_out[:]])
        VG = sing.tile([1, 2], F32)
        nc.sync.dma_start(out=VG, in_=cc2_out[:].rearrange("(o c) -> o c", o=1))
        GSC = sing.tile([1, 1], F32)
        nc.vector.tensor_scalar_mul(GSC, VG[0:1, 0:1], invN)
        nc.scalar.activation(out=GSC, in_=GSC, func=AF.Sqrt, bias=BEPSV[:])
        nc.vector.reciprocal(GSC, GSC)
        nc.vector.tensor_mul(GSC, GSC, GAM)
        G56 = sing.tile([56, 1], F32)
        nc.gpsimd.partition_broadcast(G56, GSC)

        # ================= PHASE 3 =================
        # gf = g*f ; vn = sqrt(max(gf^2*stsq, eps)); w2 = gf*sinh(vn)/vn
        nc.vector.tensor_scalar(out=PSA, in0=FPS, scalar1=G56[:, 0:1],
                                scalar2=None, op0=OP.mult)          # gf
        nc.vector.tensor_mul(PSB, PSA, PSA)
        nc.vector.tensor_mul(PSB, PSB, STSQ)
        nc.vector.tensor_scalar_max(PSB, PSB, EPS)
        VN = TPS  # T no longer needed past this point (TPB holds bf16 copy)
        nc.scalar.activation(out=VN, in_=PSB, func=AF.Sqrt)
        EX = W0DOT
        nc.scalar.activation(out=EX, in_=VN, func=AF.Exp)
        EIX = YSQ1
        nc.vector.reciprocal(EIX, EX)
        nc.vector.tensor_sub(PSB, EX, EIX)                          # 2*sinh
        nc.vector.reciprocal(PSC, VN)
        nc.vector.tensor_mul(PSB, PSB, PSC)
        nc.vector.tensor_scalar_mul(PSB, PSB, 0.5)                  # sinh/vn
        W2 = MUDOT
        nc.vector.tensor_mul(W2, PSA, PSB)
        nc.vector.tensor_scalar_mul(W2B, W2, 1.0)                   # cast bf16
        nc.sync.dma_start(out=HTW[32:33, :], in_=W2B[:])

        for band in range(NBANDS):
            b, rb = divmod(band, BANDS_PER_IMG)
            gsl = bass.ts(band, GROUPS_PER_BAND)
            OUTB = outp.tile([M, GROUPS_PER_BAND, GROUP_PX], BF16, tag="outb")
            STG_C = stgp.tile([1, GROUPS_PER_BAND, GROUP_PX], F32, tag="stgc")
            for k in range(GROUPS_PER_BAND):
                g = band * GROUPS_PER_BAND + k
                cols = bass.ts(g, GROUP_PX)
                w2p = psy.tile([128, GROUP_PX], F32, tag="psy")
                nc.tensor.matmul(w2p[0:M, :], lhsT=ONEROW[32:33, :],
                                 rhs=HTW[32:33, cols], start=True, stop=True)
                nc.vector.scalar_tensor_tensor(out=OUTB[:, k, :],
                                               in0=YCM[0:M, cols],
                                               scalar=0.0, in1=w2p[0:M, :],
                                               op0=OP.max, op1=OP.mult)
                sq_o = scrp.tile([M, GROUP_PX], BF16, tag="ysqscr")
                nc.scalar.activation(out=sq_o, in_=OUTB[:, k, :], func=AF.Square)
                ps2 = pss.tile([2, GROUP_PX], F32, tag="pss")
                nc.tensor.matmul(ps2[0:1, :], lhsT=ONESB, rhs=sq_o[:],
                                 start=True, stop=True)
                nc.vector.tensor_copy(out=STG_C[:, k, :], in_=ps2[0:1, :])
            row0 = rb * BAND_ROWS
            if band < NBANDS - 1:
                nc.gpsimd.dma_start(
                    out=out_d[b, 1:COUT, row0:row0 + BAND_ROWS, :],
                    in_=OUTB[:].rearrange("p k (r c) -> p (k r) c",
                                          r=ROWS_PER_GROUP))
            else:
                for k in range(GROUPS_PER_BAND):
                    r0 = row0 + k * ROWS_PER_GROUP
                    nc.gpsimd.dma_start(
                        out=out_d[b, 1:COUT, r0:r0 + ROWS_PER_GROUP, :],
                        in_=OUTB[:, k, :].rearrange("p (r c) -> p r c",
                                                    r=ROWS_PER_GROUP))
            nc.sync.dma_start(out=RSQ2[gsl, :], in_=STG_C[0:1, :, :])

        # rt = sqrt(1 + sum rs^2) -> channel 0 plane
        RTB = sing.tile([NGROUPS, GROUP_PX], BF16)
        nc.scalar.activation(out=RTB, in_=RSQ2, func=AF.Sqrt, bias=1.0)
        nc.sync.dma_start(out=out_d[:, 0, :, :], in_=RTB)

    nc.compile()
    return nc


def _prep_consts(W):
    W = np.asarray(W, np.float32)
    w9 = np.zeros((128, 9, 128), np.float32)
    for wi in range(9):
        w9[0:S, wi, 0:M] = W[:, 1 + wi * S:1 + (wi + 1) * S].T
        w9[64:64 + S, wi, 127] = 1.0
    redw = np.zeros((M, 3), np.float32)
    redw[:, 0] = W[:, 0]
    redw[:, 2] = 1.0
    lr1 = np.zeros((2, 128), np.float32)
    lr1[1, 0:M] = W[:, 0]
    c_w0sq = float(np.float32((W[:, 0].astype(np.float64) ** 2).sum()))
    bf = ml_dtypes.bfloat16
    return (w9.reshape(128, 9 * 128).astype(bf), redw,
            redw.astype(bf), lr1.astype(bf), c_w0sq)


def _prep_x(x):
    bf = ml_dtypes.bfloat16
    x = np.asarray(x, np.float32)
    B = x.shape[0]
    xb = np.zeros((B, 64, HP, WP), bf)
    xb[:, 0:S, 1:1 + H, 1:1 + W] = x[:, 1:].astype(bf)
    xq = np.zeros((B, 64, HP, WP), bf)
    xq[:, 0:S, 1:1 + H, 1:1 + W] = (
        xb[:, 0:S, 1:1 + H, 1:1 + W].astype(np.float32) ** 2).astype(bf)
    return xb, xq


def _in_maps(x, W, gamma):
    w9b, redw, redwb, lr1b, c_w0sq = _prep_consts(W)
    if "nc" not in _CACHE:
        _CACHE["c_w0sq"] = c_w0sq
        _CACHE["nc"] = _build_nc()
    xb, xq = _prep_x(x)
    maps = []
    for c in range(NCORES):
        sl = slice(c * B_LOC, (c + 1) * B_LOC)
        maps.append({
            "xb": xb[sl], "xq": xq[sl],
            "w9": w9b, "redw": redw, "redwb": redwb, "lr1i": lr1b,
            "gamma": np.asarray(gamma, np.float32),
        })
    return _CACHE["nc"], maps


def kernel(x, W, gamma, beta):
    gamma = np.asarray(gamma, np.float32)
    beta = np.asarray(beta, np.float32)
    assert abs(float(beta[0]) - 1.0) < 1e-6 and np.all(np.abs(beta[1:]) < 1e-6), \
        "kernel specialized for beta == Lorentz origin"
    assert float(gamma[0]) > 0.0
    nc, maps = _in_maps(x, W, gamma)
    res = run_bass_kernel_spmd(nc, maps, list(range(NCORES)))
    out = np.concatenate([np.asarray(res.results[c]["out"]).astype(np.float32)
                          for c in range(NCORES)], axis=0)
    return out


def run_traced(inputs, tmpdir=None):
    """Run with NTFF tracing; returns (exec_time_ns, BassKernelResults)."""
    nc, maps = _in_maps(inputs["x"], inputs["W"], inputs["gamma"])
    res = run_bass_kernel_spmd(nc, maps, list(range(NCORES)),
                               trace=True, tmpdir=tmpdir)
    return res.exec_time_ns, res


if __name__ == "__main__":
    rng = np.random.default_rng(0)
    x = rng.standard_normal((B_GLOB, CIN, H, W), dtype=np.float32)
    W_ = (rng.standard_normal((M, D), dtype=np.float32) / np.sqrt(D)).astype(np.float32)
    gamma = np.ones((1,), np.float32)
    beta = np.zeros((COUT,), np.float32); beta[0] = 1.0
    out = kernel(x=x, W=W_, gamma=gamma, beta=beta)
    print("out", out.shape, out.dtype, np.abs(out).max())
